# revision 23
# baseline (speedup 1.0000x reference)
"""Trainium2 Bass kernel for LoRA linear: y = x @ (W + 2*B@A).T + b.

Full inputs: x (8, 2048, 2048) f32, W (2048, 2048) f32, b (2048,) f32,
B (2048, 16) f32, A (16, 2048) f32.  Output (8, 2048, 2048) f32.

Sharding: data-parallel over the batch dim — core i computes
y[i] = x[i] @ w.T + b with the merged weight w = W + 2*B@A.

Per-core kernel (bf16 TensorEngine compute, f32 accumulate):
  phase 0: cast-DMA A/B to bf16, build 2*B.T via PE transposes,
           broadcast bias, build bf16 identity.
  phase 1: build wT[d, o] = bf16(W.T) + A.T @ (2B).T — bf16 PE transposes
           of cast-DMA'd W tiles (ScalarE evicts PSUM->SBUF), rank-16
           bf16 matmul delta in f32 PSUM added in-place by VectorE.
  phase 2: per 128-row x tile: bf16 PE transposes of the cast-DMA'd
           x tile (ScalarE evicts), then 16x [128,128]x[128,512] bf16
           matmuls per output bank, VectorE adds the bias during
           PSUM->SBUF eviction, DMA out.
"""

import numpy as np

import concourse.bacc as bacc
import concourse.mybir as mybir
import concourse.tile as tile
from concourse import masks
from concourse.bass_utils import run_bass_kernel_spmd
from concourse.tile_rust import add_dep_helper

N_CORES = 8
BATCH, S, D = 8, 2048, 2048
RANK = 16
SCALE = 2.0  # alpha / rank = 32 / 16
P = 128  # partitions
FREE = 512  # f32 elems per PSUM bank
ND = D // P  # 16 contraction tiles
NS = S // P  # 16 row tiles per core
NO = D // FREE  # 4 output banks per row tile
NG = ND // 4  # 4 transpose groups (4x 128-col transposes per PSUM bank)

F32 = mybir.dt.float32
BF16 = mybir.dt.bfloat16


def build_nc():
    nc = bacc.Bacc(
        "TRN2", target_bir_lowering=False, debug=False, num_devices=N_CORES
    )
    x_d = nc.dram_tensor("x", [S, D], F32, kind="ExternalInput").ap()
    W_d = nc.dram_tensor("W", [D, D], F32, kind="ExternalInput").ap()
    b_d = nc.dram_tensor("b", [D], F32, kind="ExternalInput").ap()
    B_d = nc.dram_tensor("B", [D, RANK], F32, kind="ExternalInput").ap()
    A_d = nc.dram_tensor("A", [RANK, D], F32, kind="ExternalInput").ap()
    out_d = nc.dram_tensor("out", [S, D], F32, kind="ExternalOutput").ap()
    # bf16 scratch holding the merged weight w = W + 2*B@A, row-major [o, d]
    Wb_d = nc.dram_tensor("Wb", [D, D], BF16).ap()

    with tile.TileContext(nc) as tc:
        with (
            tc.tile_pool(name="singles", bufs=1) as singles,
            tc.tile_pool(name="wt", bufs=1) as wtp,
        ):
            ident = singles.tile([P, P], BF16)
            masks.make_identity(nc, ident[:])

            A_sb = singles.tile([RANK, D], BF16)
            nc.gpsimd.dma_start(out=A_sb[:], in_=A_d[:])

            # 2 * B.T: cast-load B as [128, (t, r)], PE-transpose, scale
            B2T = singles.tile([RANK, D], BF16)
            Bs = singles.tile([P, ND * RANK], BF16)
            nc.gpsimd.dma_start(
                out=Bs[:], in_=B_d.rearrange("(t p) r -> p t r", p=P)
            )

            # bias replicated across all 128 partitions (needed late —
            # keep it behind A/B in the SWDGE queue)
            bb = singles.tile([P, D], F32)
            nc.gpsimd.dma_start(out=bb[:], in_=b_d[None, :].broadcast_to([P, D]))

            # merged transposed weight, bf16: wT[p, dt, o] = w[o, dt*128+p]
            wT = wtp.tile([P, ND, D], BF16)

            with (
                tc.tile_pool(name="wrow", bufs=3) as wrowp,
                tc.tile_pool(name="w16", bufs=3) as w16p,
                tc.tile_pool(name="xstage", bufs=3) as xstage,
                tc.tile_pool(name="xTp", bufs=9) as xTp,
                tc.tile_pool(name="yout", bufs=2) as youtp,
                tc.tile_pool(name="dpsum", bufs=4, space="PSUM") as dpsum,
                tc.tile_pool(name="tpsum", bufs=2, space="PSUM") as tpsum,
                tc.tile_pool(name="gpsum", bufs=2, space="PSUM") as gpsum,
            ):
                # 2*B.T from the staged B tiles (shares the delta psum slots)
                for g in range(NG):
                    bps = dpsum.tile([RANK, 4 * P], BF16, tag="dp")
                    for j in range(4):
                        t = 4 * g + j
                        nc.tensor.matmul(
                            bps[:, j * P : (j + 1) * P],
                            Bs[:, t * RANK : (t + 1) * RANK],
                            ident[:],
                            is_transpose=True,
                            start=(j == 0),
                            stop=(j == 3),
                        )
                    nc.vector.tensor_scalar_mul(
                        B2T[:, g * 4 * P : (g + 1) * 4 * P], bps[:], SCALE
                    )

                # ---- merged-weight build ----
                # Per 128-row block of W: load f32 rows, compute the rank-16
                # LoRA delta in natural [o, d] orientation on the PE
                # (delta = B2T[:, rows].T @ A), merge + cast on the DVE
                # (w16 = bf16(wrow + delta)), store the bf16 merged rows to
                # DRAM.  Then 16 DMA-xbar transposes produce wT directly.
                def w_chain(ot):
                    hw = nc.sync if ot % 2 == 0 else nc.scalar
                    wrow = wrowp.tile([P, D], F32, tag="wrow")
                    hw.dma_start(out=wrow[:], in_=W_d[ot * P : (ot + 1) * P, :])
                    w16 = w16p.tile([P, D], BF16, tag="w16")
                    for g in range(NG):
                        dp = dpsum.tile([P, FREE], F32, tag="dp")
                        nc.tensor.matmul(
                            dp[:],
                            B2T[:, ot * P : (ot + 1) * P],
                            A_sb[:, g * FREE : (g + 1) * FREE],
                            start=True,
                            stop=True,
                        )
                        nc.vector.tensor_add(
                            w16[:, g * FREE : (g + 1) * FREE],
                            dp[:],
                            wrow[:, g * FREE : (g + 1) * FREE],
                        )
                    return hw.dma_start(
                        out=Wb_d[ot * P : (ot + 1) * P, :], in_=w16[:]
                    )

                def load_and_transpose_x(st):
                    xs = xstage.tile([P, D], BF16, tag="xs")
                    nc.gpsimd.dma_start(
                        out=xs[:], in_=x_d[st * P : (st + 1) * P, :]
                    )
                    xT = xTp.tile([P, ND, P], BF16, tag="xT")
                    for g in range(NG):
                        tp = tpsum.tile([P, 4 * P], BF16, tag="tp")
                        for j in range(4):
                            dt = 4 * g + j
                            nc.tensor.matmul(
                                tp[:, j * P : (j + 1) * P],
                                xs[:, dt * P : (dt + 1) * P],
                                ident[:],
                                is_transpose=True,
                                start=(j == 0),
                                stop=(j == 3),
                            )
                        nc.scalar.copy(xT[:, 4 * g : 4 * (g + 1), :], tp[:])
                    return xT

                store_insts = [w_chain(ot) for ot in range(ND)]
                # All xbar transposes go on ONE HWDGE queue: concurrent
                # transposes on different queues corrupt each other (shared
                # xbar state); same-queue concurrency is safe.  Each
                # row-half transpose depends only on its half of the stores
                # so the first half starts while the second half still loads.
                HALF = D // 2
                for h in range(2):
                    half_stores = store_insts[h * ND // 2 : (h + 1) * ND // 2]
                    for dt in range(ND):
                        t_inst = nc.sync.dma_start_transpose(
                            out=wT[:, dt, h * HALF : (h + 1) * HALF],
                            in_=Wb_d[h * HALF : (h + 1) * HALF,
                                     dt * P : (dt + 1) * P],
                        )
                        for s_inst in half_stores:
                            add_dep_helper(t_inst.ins, s_inst.ins, reason="Wb RAW")

                PRE = 8  # x row-tiles transposed ahead of the GEMM
                xTs = [load_and_transpose_x(st) for st in range(PRE)]

                # ---- main loop: y = x @ wT + b ----
                for st in range(NS):
                    if st + PRE < NS:
                        xTs.append(load_and_transpose_x(st + PRE))
                    xT = xTs[st]
                    ys = youtp.tile([P, D], F32)
                    for oc in range(NO):
                        gp = gpsum.tile([P, FREE], F32)
                        for dt in range(ND):
                            nc.tensor.matmul(
                                gp[:],
                                xT[:, dt, :],
                                wT[:, dt, oc * FREE : (oc + 1) * FREE],
                                start=(dt == 0),
                                stop=(dt == ND - 1),
                            )
                        nc.vector.tensor_add(
                            ys[:, oc * FREE : (oc + 1) * FREE],
                            gp[:],
                            bb[:, oc * FREE : (oc + 1) * FREE],
                        )
                    # y stores on the sync queue: the scalar HWDGE queue must
                    # stay quiet while transposes might still be in flight
                    nc.sync.dma_start(out=out_d[st * P : (st + 1) * P, :], in_=ys[:])

    nc.compile()
    return nc


_NC_CACHE = None


def _get_nc():
    global _NC_CACHE
    if _NC_CACHE is None:
        _NC_CACHE = build_nc()
    return _NC_CACHE


def make_in_maps(x, W, b, B, A):
    x = np.ascontiguousarray(x, dtype=np.float32)
    W = np.ascontiguousarray(W, dtype=np.float32)
    b = np.ascontiguousarray(b, dtype=np.float32)
    B = np.ascontiguousarray(B, dtype=np.float32)
    A = np.ascontiguousarray(A, dtype=np.float32)
    return [
        {"x": x[i], "W": W, "b": b, "B": B, "A": A} for i in range(N_CORES)
    ]


def run(inputs, **spmd_kwargs):
    """Run the SPMD kernel; returns (output, BassKernelResults)."""
    nc = _get_nc()
    in_maps = make_in_maps(**inputs)
    res = run_bass_kernel_spmd(nc, in_maps, core_ids=list(range(N_CORES)), **spmd_kwargs)
    out = np.stack([res.results[i]["out"] for i in range(N_CORES)]).astype(np.float32)
    return out, res


def kernel(x, W, b, B, A):
    out, _ = run({"x": x, "W": W, "b": b, "B": B, "A": A})
    return out


# revision 28
# speedup vs baseline: 1.0811x; 1.0811x over previous
"""Trainium2 Bass kernel for LoRA linear: y = x @ (W + 2*B@A).T + b.

Full inputs: x (8, 2048, 2048) f32, W (2048, 2048) f32, b (2048,) f32,
B (2048, 16) f32, A (16, 2048) f32.  Output (8, 2048, 2048) f32.

Sharding: data-parallel over the batch dim — core i computes
y[i] = x[i] @ w.T + b with the merged weight w = W + 2*B@A.

Per-core kernel (bf16 TensorEngine compute, f32 accumulate):
  phase 0: cast-DMA A/B to bf16, build 2*B.T via PE transposes,
           broadcast bias, build bf16 identity.
  phase 1: build wT[d, o] = bf16(W.T) + A.T @ (2B).T — bf16 PE transposes
           of cast-DMA'd W tiles (ScalarE evicts PSUM->SBUF), rank-16
           bf16 matmul delta in f32 PSUM added in-place by VectorE.
  phase 2: per 128-row x tile: bf16 PE transposes of the cast-DMA'd
           x tile (ScalarE evicts), then 16x [128,128]x[128,512] bf16
           matmuls per output bank, VectorE adds the bias during
           PSUM->SBUF eviction, DMA out.
"""

import numpy as np

import concourse.bacc as bacc
import concourse.mybir as mybir
import concourse.tile as tile
from concourse import masks
from concourse.bass_utils import run_bass_kernel_spmd
from concourse.tile_rust import add_dep_helper

N_CORES = 8
BATCH, S, D = 8, 2048, 2048
RANK = 16
SCALE = 2.0  # alpha / rank = 32 / 16
P = 128  # partitions
FREE = 512  # f32 elems per PSUM bank
ND = D // P  # 16 contraction tiles
NS = S // P  # 16 row tiles per core
NO = D // FREE  # 4 output banks per row tile
NG = ND // 4  # 4 transpose groups (4x 128-col transposes per PSUM bank)

F32 = mybir.dt.float32
BF16 = mybir.dt.bfloat16


def build_nc():
    nc = bacc.Bacc(
        "TRN2", target_bir_lowering=False, debug=False, num_devices=N_CORES
    )
    x_d = nc.dram_tensor("x", [S, D], F32, kind="ExternalInput").ap()
    W_d = nc.dram_tensor("W", [D, D], F32, kind="ExternalInput").ap()
    b_d = nc.dram_tensor("b", [D], F32, kind="ExternalInput").ap()
    B_d = nc.dram_tensor("B", [D, RANK], F32, kind="ExternalInput").ap()
    A_d = nc.dram_tensor("A", [RANK, D], F32, kind="ExternalInput").ap()
    out_d = nc.dram_tensor("out", [S, D], F32, kind="ExternalOutput").ap()
    # bf16 scratch holding the merged weight w = W + 2*B@A, row-major [o, d]
    Wb_d = nc.dram_tensor("Wb", [D, D], BF16).ap()

    with tile.TileContext(nc) as tc:
        with (
            tc.tile_pool(name="singles", bufs=1) as singles,
            tc.tile_pool(name="wt", bufs=1) as wtp,
        ):
            ident = singles.tile([P, P], BF16)
            masks.make_identity(nc, ident[:])

            A_sb = singles.tile([RANK, D], BF16)
            nc.gpsimd.dma_start(out=A_sb[:], in_=A_d[:])

            # 2 * B.T: cast-load B as [128, (t, r)], PE-transpose, scale
            B2T = singles.tile([RANK, D], BF16)
            Bs = singles.tile([P, ND * RANK], BF16)
            nc.gpsimd.dma_start(
                out=Bs[:], in_=B_d.rearrange("(t p) r -> p t r", p=P)
            )

            # bias replicated across all 128 partitions (needed late —
            # keep it behind A/B in the SWDGE queue)
            bb = singles.tile([P, D], F32)
            nc.gpsimd.dma_start(out=bb[:], in_=b_d[None, :].broadcast_to([P, D]))

            # merged transposed weight, bf16: wT[p, dt, o] = w[o, dt*128+p]
            wT = wtp.tile([P, ND, D], BF16)

            with (
                tc.tile_pool(name="wrow", bufs=3) as wrowp,
                tc.tile_pool(name="w16", bufs=3) as w16p,
                tc.tile_pool(name="xstage", bufs=4) as xstage,
                tc.tile_pool(name="xTp", bufs=5) as xTp,
                tc.tile_pool(name="yout", bufs=2) as youtp,
                tc.tile_pool(name="dpsum", bufs=4, space="PSUM") as dpsum,
                tc.tile_pool(name="tpsum", bufs=2, space="PSUM") as tpsum,
                tc.tile_pool(name="gpsum", bufs=2, space="PSUM") as gpsum,
            ):
                # 2*B.T from the staged B tiles (shares the delta psum slots)
                for g in range(NG):
                    bps = dpsum.tile([RANK, 4 * P], BF16, tag="dp")
                    for j in range(4):
                        t = 4 * g + j
                        nc.tensor.matmul(
                            bps[:, j * P : (j + 1) * P],
                            Bs[:, t * RANK : (t + 1) * RANK],
                            ident[:],
                            is_transpose=True,
                            start=(j == 0),
                            stop=(j == 3),
                        )
                    nc.vector.tensor_scalar_mul(
                        B2T[:, g * 4 * P : (g + 1) * 4 * P], bps[:], SCALE
                    )

                # ---- merged-weight build ----
                # Per 128-row block of W: load f32 rows, compute the rank-16
                # LoRA delta in natural [o, d] orientation on the PE
                # (delta = B2T[:, rows].T @ A), merge + cast on the DVE
                # (w16 = bf16(wrow + delta)), store the bf16 merged rows to
                # DRAM.  Then 16 DMA-xbar transposes produce wT directly.
                def w_chain(ot):
                    # loads on the scalar HWDGE queue, stores (+ transposes,
                    # later) on sync — mixing them in one ring head-of-line
                    # blocks loads behind stores that wait on the DVE merge
                    wrow = wrowp.tile([P, D], F32, tag="wrow")
                    nc.scalar.dma_start(
                        out=wrow[:], in_=W_d[ot * P : (ot + 1) * P, :]
                    )
                    w16 = w16p.tile([P, D], BF16, tag="w16")
                    dps = [
                        dpsum.tile([P, FREE], F32, tag="dp", name=f"dp{ot}_{g}")
                        for g in range(NG)
                    ]
                    for g in range(NG):
                        nc.tensor.matmul(
                            dps[g][:],
                            B2T[:, ot * P : (ot + 1) * P],
                            A_sb[:, g * FREE : (g + 1) * FREE],
                            start=True,
                            stop=True,
                        )
                    for g in range(NG):
                        nc.vector.tensor_add(
                            w16[:, g * FREE : (g + 1) * FREE],
                            dps[g][:],
                            wrow[:, g * FREE : (g + 1) * FREE],
                        )
                    return nc.sync.dma_start(
                        out=Wb_d[ot * P : (ot + 1) * P, :], in_=w16[:]
                    )

                def load_and_transpose_x(st):
                    xs = xstage.tile([P, D], BF16, tag="xs")
                    nc.gpsimd.dma_start(
                        out=xs[:], in_=x_d[st * P : (st + 1) * P, :]
                    )
                    xT = xTp.tile([P, ND, P], BF16, tag="xT")
                    # 8 transposes per bf16 PSUM bank, one ScalarE evict each
                    for g in range(2):
                        tp = tpsum.tile([P, 8 * P], BF16, tag="tp")
                        for j in range(8):
                            dt = 8 * g + j
                            nc.tensor.matmul(
                                tp[:, j * P : (j + 1) * P],
                                xs[:, dt * P : (dt + 1) * P],
                                ident[:],
                                is_transpose=True,
                                start=(j == 0),
                                stop=(j == 7),
                            )
                        nc.scalar.copy(xT[:, 8 * g : 8 * (g + 1), :], tp[:])
                    return xT

                store_insts = [w_chain(ot) for ot in range(ND)]
                # All xbar transposes go on ONE HWDGE queue: concurrent
                # transposes on different queues corrupt each other (shared
                # xbar state); same-queue concurrency is safe.  Each
                # row-half transpose depends only on its half of the stores
                # so the first half starts while the second half still loads.
                HALF = D // 2
                for h in range(2):
                    half_stores = store_insts[h * ND // 2 : (h + 1) * ND // 2]
                    for dt in range(ND):
                        t_inst = nc.sync.dma_start_transpose(
                            out=wT[:, dt, h * HALF : (h + 1) * HALF],
                            in_=Wb_d[h * HALF : (h + 1) * HALF,
                                     dt * P : (dt + 1) * P],
                        )
                        for s_inst in half_stores:
                            add_dep_helper(t_inst.ins, s_inst.ins, reason="Wb RAW")

                PRE = 4  # x row-tiles transposed ahead of the GEMM
                xTs = [load_and_transpose_x(st) for st in range(PRE)]

                # ---- main loop: y = x @ wT + b ----
                for st in range(NS):
                    if st + PRE < NS:
                        xTs.append(load_and_transpose_x(st + PRE))
                    xT = xTs[st]
                    ys = youtp.tile([P, D], F32)
                    for oc in range(NO):
                        gp = gpsum.tile([P, FREE], F32)
                        for dt in range(ND):
                            nc.tensor.matmul(
                                gp[:],
                                xT[:, dt, :],
                                wT[:, dt, oc * FREE : (oc + 1) * FREE],
                                start=(dt == 0),
                                stop=(dt == ND - 1),
                            )
                        nc.vector.tensor_add(
                            ys[:, oc * FREE : (oc + 1) * FREE],
                            gp[:],
                            bb[:, oc * FREE : (oc + 1) * FREE],
                        )
                    # y stores on the sync queue: the scalar HWDGE queue must
                    # stay quiet while transposes might still be in flight
                    nc.sync.dma_start(out=out_d[st * P : (st + 1) * P, :], in_=ys[:])

    nc.compile()
    return nc


_NC_CACHE = None


def _get_nc():
    global _NC_CACHE
    if _NC_CACHE is None:
        _NC_CACHE = build_nc()
    return _NC_CACHE


def make_in_maps(x, W, b, B, A):
    x = np.ascontiguousarray(x, dtype=np.float32)
    W = np.ascontiguousarray(W, dtype=np.float32)
    b = np.ascontiguousarray(b, dtype=np.float32)
    B = np.ascontiguousarray(B, dtype=np.float32)
    A = np.ascontiguousarray(A, dtype=np.float32)
    return [
        {"x": x[i], "W": W, "b": b, "B": B, "A": A} for i in range(N_CORES)
    ]


def run(inputs, **spmd_kwargs):
    """Run the SPMD kernel; returns (output, BassKernelResults)."""
    nc = _get_nc()
    in_maps = make_in_maps(**inputs)
    res = run_bass_kernel_spmd(nc, in_maps, core_ids=list(range(N_CORES)), **spmd_kwargs)
    out = np.stack([res.results[i]["out"] for i in range(N_CORES)]).astype(np.float32)
    return out, res


def kernel(x, W, b, B, A):
    out, _ = run({"x": x, "W": W, "b": b, "B": B, "A": A})
    return out
